# revision 24
# baseline (speedup 1.0000x reference)
"""Causal multi-head attention (b=2, n=2048, d=768, 12 heads) on 8 TRN2 NeuronCores.

Sharding: batch x head-group. Core c handles batch c//4 and heads 3*(c%4) .. 3*(c%4)+2.
Each core gets xT = x[b].T plus W.T column slices for its 3 heads, computes the
unnormalized attention output (transposed) plus softmax denominators; the host
divides, transposes, and concatenates slabs into the full [2, 2048, 768].

v3: attention runs the PE in 64x128 row-tiled mode (tile_position) so the two
64-dim head computations occupy disjoint array halves and run CONCURRENTLY
(delta-start ~4ns per the TRN2 tiling docs):
  scores jt (heads 0,1): T0 = k0[0:64].T @ q[0:64], T8 = k1[64:128].T @ q[64:128]
    -> one wall covers both heads (K=64 padding waste eliminated)
  scores head 2: consecutive j-tiles paired via duplicated k2/q2 rows [64:128]
  AV: K=128 keys split into lo/hi halves on T0/T8 -> two psum accumulators
    avA (keys 0:64) + avB (keys 64:128); combined during the PSUM->SBUF
    evacuation as one DVE tensor_add (the copy was needed anyway)
  diag 128-blocks masked post-exp with a 0/1 bf16 triangular multiply (DVE
  TensorTensor cannot touch PSUM per the BIR verifier)
  p = exp(scores) unshifted (max causal score ~66 fits fp32), bf16; v_nat bf16

The exp stream on ScalarE (1 elem/cyc/lane @1.2GHz) outweighs the row-tiled
TensorE attention walls, so span s+1's projection work (ACT-free) is emitted
interleaved into span s's attention stream in 4-op bursts. Bursts are kept to
an EVEN number of psum-ring allocations mid-loop (ring parity keeps the Tile
WAR tracking sound: a slot is only re-allocated after the previous occupant's
readers were emitted); odd leftovers flush at phase boundaries.

Perf-critical TRN2 facts (measured / from trainium-docs):
  - PSUM bank = 512 fp32: every matmul output must stay within one bank
  - row-tiled concurrent MMs: span ~ MM_dur + 4ns/tile; LDWEIGHTS overlaps
    in-flight MMs of the other row group; row tiles must hit different banks
  - f32r 1.06 cyc/row @2.4GHz after ~3.4us HAM warmup; ~165ns fixed/MM
  - f32r tiles can only be produced by DVE/ACT compute ops
  - GPSIMD has no PSUM port; DVE handles PSUM operands at 1x
"""
import sys

if "/opt/trn_rl_repo" not in sys.path:
    sys.path.insert(0, "/opt/trn_rl_repo")

from contextlib import ExitStack

import numpy as np

import concourse.bass as bass
import concourse.tile as tile
from concourse import bacc, mybir, bass_utils
from concourse.masks import make_identity

F32 = mybir.dt.float32
F32R = mybir.dt.float32r
BF16 = mybir.dt.bfloat16

P = 128
H = 64                       # half partition
SPAN = 512
HD = 64

B, N, D, NH = 2, 2048, 768, 12
HL = 3                       # heads per core
DL = HL * HD                 # 192
N_CORES = 8
KT = D // P                  # 6 contraction chunks
KH = KT // 2                 # kt per x/w half
NS = N // SPAN               # 4 spans
NT = N // P                  # 16 j-tiles
CPS = SPAN // P              # 4 chunks per span

DT_PROJ = F32R               # x, W, qT/kT/vT
DT_P = BF16                  # p = exp(scores), v_nat
MASK_VAL = -30000.0
WARMUP_N = 26
BURST = 4                    # proj ops per mid-loop insertion (even!)
EXP = mybir.ActivationFunctionType.Exp


def _build(nc, tc, dt_proj, dt_p):
    # host pre-packs: xt[p, (ns, kth, ktl, c)], wc[p, (kt, m)] with
    # m = packed weight columns [q01 | k01 | v01 | k2+v2 | q2]
    xt = nc.dram_tensor("xt", [P, N * KT], dt_proj, kind="ExternalInput").ap()
    wc = nc.dram_tensor("wc", [P, KT * 3 * DL], dt_proj,
                        kind="ExternalInput").ap()
    o = nc.dram_tensor("o", [HL * (HD + 1), N], F32, kind="ExternalOutput").ap()

    with ExitStack() as ctx:
        pool = lambda name, bufs, **kw: ctx.enter_context(
            tc.tile_pool(name=name, bufs=bufs, **kw))
        const_pool = pool("const", 1)
        xpool = pool("x", 2 * NS)
        wpool = pool("w", 2)
        qk_pool = pool("qk", NS)
        vnat_pool = pool("vnat", 1)
        ppool = pool("p", 6)
        osb_pool = pool("osb", 3)
        ps = pool("ps", 2, space="PSUM")        # [128,1024] pair tiles: 2x2 banks
        ps_av = pool("ps_av", 4, space="PSUM")  # [65,512] accumulators: 4x1 bank

        ident = const_pool.tile([P, P], F32)
        make_identity(nc, ident[:])
        ident_r = const_pool.tile([P, P], dt_proj)
        nc.vector.tensor_copy(ident_r[:], ident[:])
        # multiplicative causal mask for [key-partition, query-col] diag blocks:
        # 1 where key <= query, 0 where key > query; bf16 so the post-exp p
        # multiply runs at the DVE 16-bit rate (TT cannot touch PSUM)
        tri16 = const_pool.tile([P, P], dt_p)
        nc.gpsimd.memset(tri16[:], 0.0)
        nc.gpsimd.affine_select(
            out=tri16[:], in_=tri16[:], compare_op=mybir.AluOpType.is_gt,
            fill=1.0, base=0, pattern=[[-1, P]], channel_multiplier=1)
        ones32 = const_pool.tile([P, 2 * NT], F32)
        nc.gpsimd.memset(ones32[:], 1.0)
        zeros = const_pool.tile([P, SPAN], F32)
        nc.gpsimd.memset(zeros[:], 0.0)
        zeros_r = const_pool.tile([P, SPAN], dt_proj)
        nc.vector.tensor_copy(zeros_r[:], zeros[:])

        # ---- DMA inputs: w halves + 8 x chunks (span, kt-half) ----
        x_tiles = [xpool.tile([P, KH * SPAN], dt_proj, tag="x", name=f"x{i}")
                   for i in range(2 * NS)]

        def x_slice(ns, kt):
            t = x_tiles[2 * ns + kt // KH]
            b = (kt % KH) * SPAN
            return t[:, b:b + SPAN]

        def dma_x(ns, half):
            w = KH * SPAN
            i = 2 * ns + half
            nc.sync.dma_start(x_tiles[i][:], xt[:, i * w:(i + 1) * w])

        WCW = 3 * DL
        w_half = [wpool.tile([P, KH * WCW], dt_proj, tag=f"w{i}", name=f"w{i}")
                  for i in range(2)]

        def w_slice(kt, c0, c1):
            t = w_half[kt // KH]
            b = (kt % KH) * WCW
            return t[:, b + c0:b + c1]

        def dma_w(half):
            w = KH * WCW
            nc.sync.dma_start(w_half[half][:], wc[:, half * w:(half + 1) * w])

        dma_w(0)
        dma_x(0, 0)
        dma_x(0, 1)
        dma_w(1)
        for ns in range(1, NS):
            dma_x(ns, 0)
            dma_x(ns, 1)

        # ---- per-span q/k/v tiles ----
        # q01/k01: rows 0:64 = head0 dims, 64:128 = head1 dims (natural)
        # q2d/k2d: head2 q/k duplicated into both row halves (for jt pairing)
        # vT2z: rows 64:128 = head2 v dims, rows 0:64 zero (transpose trick)
        q01 = [qk_pool.tile([P, SPAN], dt_proj, tag="q01", name=f"q01_{i}") for i in range(NS)]
        q2d = [qk_pool.tile([P, SPAN], dt_proj, tag="q2d", name=f"q2d_{i}") for i in range(NS)]
        k01 = [qk_pool.tile([P, SPAN], dt_proj, tag="k01", name=f"k01_{i}") for i in range(NS)]
        k2d = [qk_pool.tile([P, SPAN], dt_proj, tag="k2d", name=f"k2d_{i}") for i in range(NS)]
        vT01 = [qk_pool.tile([P, SPAN], dt_proj, tag="v01", name=f"v01_{i}") for i in range(NS)]
        vT2z = [qk_pool.tile([P, SPAN], dt_proj, tag="v2z", name=f"v2z_{i}") for i in range(NS)]
        for ns in range(NS):
            nc.vector.tensor_copy(vT2z[ns][0:HD, :], zeros[0:HD, :])

        # v natural layout: heads 0,1 interleaved per j-tile [v0|1|v1|1], head 2
        # separate [v2|1]; the ones column accumulates the softmax denominator.
        v_nat01 = vnat_pool.tile([P, NT * 2 * (HD + 1)], dt_p, tag="vnat01")
        v_nat2 = vnat_pool.tile([P, NT * (HD + 1)], dt_p, tag="vnat2")
        c01 = v_nat01[:].rearrange("p (t c) -> p t c", c=HD + 1)[:, :, HD]
        c2 = v_nat2[:].rearrange("p (t c) -> p t c", c=HD + 1)[:, :, HD]
        nc.vector.tensor_copy(c01, ones32[:])
        nc.vector.tensor_copy(c2, ones32[:, 0:NT])

        def vnat(h, jt):
            if h < 2:
                b = jt * 2 * (HD + 1) + h * (HD + 1)
                return v_nat01[:, b:b + HD + 1]
            b = jt * (HD + 1)
            return v_nat2[:, b:b + HD + 1]

        # ---- warmup: keep the PE busy while the first DMAs land ----
        warm = ps.tile([P, SPAN], F32, tag="ps", name="warm")
        for _ in range(WARMUP_N):
            nc.tensor.matmul(warm[:], ident_r[:], zeros_r[:],
                             start=True, stop=True)

        # ---- projections as an op-list (5 chunk ops + 8 transpose ops) ----
        m_chunks = ((0, P, "q01"), (P, P, "k01"), (2 * P, P, "v01"),
                    (3 * P, P, "k2v2"), (4 * P, HD, "q2"))

        def chunk_op(ns, moff, msz, what):
            pt = ps.tile([msz, SPAN], F32, tag="ps", name=f"pj_{ns}_{what}")
            for kt in range(KT):
                nc.tensor.matmul(
                    pt[:], w_slice(kt, moff, moff + msz), x_slice(ns, kt),
                    start=(kt == 0), stop=(kt == KT - 1))
            if what == "q01":
                nc.vector.tensor_copy(q01[ns][:], pt[:])
            elif what == "k01":
                nc.vector.tensor_copy(k01[ns][:], pt[:])
            elif what == "v01":
                nc.vector.tensor_copy(vT01[ns][:], pt[:])
            elif what == "k2v2":
                nc.vector.tensor_copy(k2d[ns][0:HD, :], pt[0:HD, :])
                nc.vector.tensor_copy(k2d[ns][HD:P, :], pt[0:HD, :])
                nc.vector.tensor_copy(vT2z[ns][HD:P, :], pt[HD:P, :])
            else:
                nc.vector.tensor_copy(q2d[ns][0:HD, :], pt[:])
                nc.vector.tensor_copy(q2d[ns][HD:P, :], pt[:])

        def transp01_op(ns, c):
            jt = ns * CPS + c
            tp = ps.tile([P, P], dt_proj, tag="ps", name=f"tp_{jt}")
            nc.tensor.transpose(tp[:], vT01[ns][:, c * P:(c + 1) * P],
                                ident_r[:])
            nc.vector.tensor_copy(
                v_nat01[:].rearrange("p (t c) -> p t c", c=HD + 1)[
                    :, 2 * jt:2 * jt + 2, 0:HD],
                tp[:].rearrange("p (t c) -> p t c", c=HD))

        def transp2_op(ns, c):
            jt = ns * CPS + c
            tp2 = ps.tile([P, P], dt_proj, tag="ps", name=f"tp2_{jt}")
            nc.tensor.transpose(tp2[:], vT2z[ns][:, c * P:(c + 1) * P],
                                ident_r[:])
            nc.vector.tensor_copy(
                v_nat2[:, jt * (HD + 1):jt * (HD + 1) + HD], tp2[:, HD:P])

        def proj_ops(ns):
            ops = [lambda a=moff, b=msz, w=what: chunk_op(ns, a, b, w)
                   for (moff, msz, what) in m_chunks]
            for c in range(CPS):
                ops.append(lambda c=c: transp01_op(ns, c))
                ops.append(lambda c=c: transp2_op(ns, c))
            return ops

        pending = []

        def drain(k):
            while k > 0 and pending:
                pending.pop(0)()
                k -= 1

        def boundary():
            drain(len(pending))

        def finalize(s, h, avA, avB):
            # DVE has one PSUM read port: copy then add (one psum src each)
            ob = osb_pool.tile([HD + 1, SPAN], F32, tag="osb", name=f"ob{s}_{h}")
            nc.vector.tensor_copy(ob[:], avA[:])
            nc.vector.tensor_add(ob[:], ob[:], avB[:])
            nc.sync.dma_start(
                o[h * (HD + 1):(h + 1) * (HD + 1),
                  s * SPAN:(s + 1) * SPAN], ob[:])

        # ---- attention, heads 0+1 fused via row-tiled concurrency ----
        def attn01(s):
            njt = CPS * (s + 1)
            avs = [ps_av.tile([HD + 1, SPAN], F32, tag="ps_av",
                              name=f"av{h}{half}_{s}")
                   for h in range(2) for half in range(2)]
            av0A, av0B, av1A, av1B = avs
            live = {}

            def emit_sc(jt):
                c_d = jt - CPS * s
                n0 = max(c_d, 0) * P
                ns_k, ck = jt // CPS, jt % CPS
                kb = k01[ns_k][:, ck * P:(ck + 1) * P]
                sc = ps.tile([P, 2 * SPAN], F32, tag="ps", name=f"sc01_{s}_{jt}")
                nc.tensor.matmul(sc[:, n0:SPAN], kb[0:H, :],
                                 q01[s][0:H, n0:SPAN],
                                 start=True, stop=True, tile_position=(0, 0))
                nc.tensor.matmul(sc[:, SPAN + n0:2 * SPAN], kb[H:P, :],
                                 q01[s][H:P, n0:SPAN],
                                 start=True, stop=True, tile_position=(H, 0))
                live[jt] = (sc, n0, c_d >= 0)

            def emit_pav(jt):
                sc, n0, diag = live.pop(jt)
                p = ppool.tile([P, 2 * SPAN], dt_p, tag="p", name=f"p01_{s}_{jt}")
                sc3 = sc[:].rearrange("q (t c) -> q t c", c=SPAN)
                p3 = p[:].rearrange("q (t c) -> q t c", c=SPAN)
                nc.scalar.activation(p3[:, :, n0:SPAN], sc3[:, :, n0:SPAN], EXP)
                if diag:
                    nc.vector.tensor_mul(p[:, n0:n0 + P], p[:, n0:n0 + P],
                                         tri16[:])
                    nc.vector.tensor_mul(
                        p[:, SPAN + n0:SPAN + n0 + P],
                        p[:, SPAN + n0:SPAN + n0 + P], tri16[:])
                # safe insertion point: every live sc tile's reader is emitted,
                # so proj ops may take any number of psum-ring slots; the burst
                # (or the pre-emitted next sc) hides the exp latency before the
                # av matmuls below
                if pending:
                    drain(BURST)
                elif jt + 1 < njt and jt + 1 not in live:
                    emit_sc(jt + 1)
                st, sp = (jt == 0), (jt == njt - 1)
                v0, v1 = vnat(0, jt), vnat(1, jt)
                nc.tensor.matmul(av0A[:, n0:SPAN], v0[0:H, :],
                                 p[0:H, n0:SPAN], start=st, stop=sp,
                                 tile_position=(0, 0))
                nc.tensor.matmul(av0B[:, n0:SPAN], v0[H:P, :],
                                 p[H:P, n0:SPAN], start=st, stop=sp,
                                 tile_position=(H, 0))
                nc.tensor.matmul(av1A[:, n0:SPAN], v1[0:H, :],
                                 p[0:H, SPAN + n0:2 * SPAN], start=st, stop=sp,
                                 tile_position=(0, 0))
                nc.tensor.matmul(av1B[:, n0:SPAN], v1[H:P, :],
                                 p[H:P, SPAN + n0:2 * SPAN], start=st, stop=sp,
                                 tile_position=(H, 0))

            for jt in range(njt):
                if jt not in live:
                    emit_sc(jt)
                emit_pav(jt)
            finalize(s, 0, av0A, av0B)
            finalize(s, 1, av1A, av1B)

        # ---- head 2: consecutive j-tiles paired across the row halves ----
        def attn2(s):
            njt = CPS * (s + 1)
            npair = njt // 2
            av2A = ps_av.tile([HD + 1, SPAN], F32, tag="ps_av", name=f"av2A_{s}")
            av2B = ps_av.tile([HD + 1, SPAN], F32, tag="ps_av", name=f"av2B_{s}")
            live = {}

            def offs(jt):
                return max(jt - CPS * s, 0) * P

            def emit_sc(i):
                je, jo = 2 * i, 2 * i + 1
                n0e, n0o = offs(je), offs(jo)
                sc = ps.tile([P, 2 * SPAN], F32, tag="ps", name=f"sc2_{s}_{i}")
                ke = k2d[je // CPS][0:H, (je % CPS) * P:(je % CPS + 1) * P]
                ko = k2d[jo // CPS][H:P, (jo % CPS) * P:(jo % CPS + 1) * P]
                nc.tensor.matmul(sc[:, n0e:SPAN], ke, q2d[s][0:H, n0e:SPAN],
                                 start=True, stop=True, tile_position=(0, 0))
                nc.tensor.matmul(sc[:, SPAN + n0o:2 * SPAN], ko,
                                 q2d[s][H:P, n0o:SPAN],
                                 start=True, stop=True, tile_position=(H, 0))
                live[i] = (sc, n0e, n0o)

            def emit_pav(i):
                je, jo = 2 * i, 2 * i + 1
                sc, n0e, n0o = live.pop(i)
                p = ppool.tile([P, 2 * SPAN], dt_p, tag="p", name=f"p2_{s}_{i}")
                if n0e == 0 and n0o == 0:
                    sc3 = sc[:].rearrange("q (t c) -> q t c", c=SPAN)
                    p3 = p[:].rearrange("q (t c) -> q t c", c=SPAN)
                    nc.scalar.activation(p3[:, :, :], sc3[:, :, :], EXP)
                else:
                    nc.scalar.activation(p[:, n0e:SPAN], sc[:, n0e:SPAN], EXP)
                    nc.scalar.activation(p[:, SPAN + n0o:2 * SPAN],
                                         sc[:, SPAN + n0o:2 * SPAN], EXP)
                if je - CPS * s >= 0:
                    nc.vector.tensor_mul(p[:, n0e:n0e + P], p[:, n0e:n0e + P],
                                         tri16[:])
                if jo - CPS * s >= 0:
                    nc.vector.tensor_mul(
                        p[:, SPAN + n0o:SPAN + n0o + P],
                        p[:, SPAN + n0o:SPAN + n0o + P], tri16[:])
                if pending:
                    drain(BURST)
                elif i + 1 < npair and i + 1 not in live:
                    emit_sc(i + 1)
                st, sp = (i == 0), (i == npair - 1)
                ve, vo = vnat(2, je), vnat(2, jo)
                nc.tensor.matmul(av2A[:, n0e:SPAN], ve[0:H, :],
                                 p[0:H, n0e:SPAN], start=st, stop=False,
                                 tile_position=(0, 0))
                nc.tensor.matmul(av2B[:, n0e:SPAN], ve[H:P, :],
                                 p[H:P, n0e:SPAN], start=st, stop=False,
                                 tile_position=(H, 0))
                nc.tensor.matmul(av2A[:, n0o:SPAN], vo[0:H, :],
                                 p[0:H, SPAN + n0o:2 * SPAN], start=False,
                                 stop=sp, tile_position=(0, 0))
                nc.tensor.matmul(av2B[:, n0o:SPAN], vo[H:P, :],
                                 p[H:P, SPAN + n0o:2 * SPAN], start=False,
                                 stop=sp, tile_position=(H, 0))

            for i in range(npair):
                if i not in live:
                    emit_sc(i)
                emit_pav(i)
            finalize(s, 2, av2A, av2B)

        # span 0 projections run standalone; later spans interleave into the
        # previous span's attention stream
        for op in proj_ops(0):
            op()
        for s in range(NS):
            pending.extend(proj_ops(s + 1) if s + 1 < NS else [])
            attn01(s)
            boundary()
            attn2(s)
            boundary()


_NC_CACHE = {}


def _get_module(dt_proj=DT_PROJ, dt_p=DT_P):
    key = (dt_proj, dt_p)
    if key not in _NC_CACHE:
        nc = bacc.Bacc("TRN2", target_bir_lowering=False, debug=False)
        with tile.TileContext(nc) as tc:
            _build(nc, tc, dt_proj, dt_p)
        nc.compile()
        _NC_CACHE[key] = nc
    return _NC_CACHE[key]


def _in_maps(x, Wq, Wk, Wv):
    maps = []
    xT = [np.ascontiguousarray(
        x[b].T.reshape(KT, P, NS, SPAN).transpose(1, 2, 0, 3).reshape(P, -1))
        for b in range(B)]
    WqT, WkT, WvT = Wq.T, Wk.T, Wv.T
    for c in range(N_CORES):
        bc, g = divmod(c, N_CORES // B)
        s0 = g * DL
        wcomb = np.concatenate([
            WqT[:, s0:s0 + P], WkT[:, s0:s0 + P], WvT[:, s0:s0 + P],
            WkT[:, s0 + P:s0 + DL], WvT[:, s0 + P:s0 + DL],
            WqT[:, s0 + P:s0 + DL]], axis=1)
        wpk = np.ascontiguousarray(
            wcomb.reshape(KT, P, 3 * DL).transpose(1, 0, 2).reshape(P, -1))
        maps.append({
            "xt": xT[bc],
            "wc": wpk,
        })
    return maps


def kernel(x, Wq, Wk, Wv, _trace=False, _tmpdir=None, **_kw):
    x = np.asarray(x, dtype=np.float32)
    Wq = np.asarray(Wq, dtype=np.float32)
    Wk = np.asarray(Wk, dtype=np.float32)
    Wv = np.asarray(Wv, dtype=np.float32)
    assert x.shape == (B, N, D) and Wq.shape == (D, D)

    nc = _get_module()
    res = bass_utils.run_bass_kernel_spmd(
        nc, _in_maps(x, Wq, Wk, Wv), core_ids=list(range(N_CORES)),
        trace=_trace, tmpdir=_tmpdir)
    out = np.empty((B, N, D), np.float32)
    for c in range(N_CORES):
        bc, g = divmod(c, N_CORES // B)
        oT = res.results[c]["o"].astype(np.float64)
        for h in range(HL):
            blk = oT[h * (HD + 1):h * (HD + 1) + HD, :]
            den = oT[h * (HD + 1) + HD, :]
            out[bc, :, g * DL + h * HD:g * DL + (h + 1) * HD] = \
                (blk / den).T.astype(np.float32)
    if _trace:
        return out, res
    return out


# revision 25
# speedup vs baseline: 1.0205x; 1.0205x over previous
"""Causal multi-head attention (b=2, n=2048, d=768, 12 heads) on 8 TRN2 NeuronCores.

Sharding: batch x head-group. Core c handles batch c//4 and heads 3*(c%4) .. 3*(c%4)+2.
Each core gets xT = x[b].T plus W.T column slices for its 3 heads, computes the
unnormalized attention output (transposed) plus softmax denominators; the host
divides, transposes, and concatenates slabs into the full [2, 2048, 768].

Per-core algorithm (everything transposed so softmax reductions ride on matmuls):
  qT/kT/vT = (W.T slice).T @ xT            TensorE, per 512-col span
  v_nat[j, m] = transpose(vT) + ones column -> stationary [128, 65] per j-tile
  per head, per 512-col i-span:
    sT[j, i] = kT_h[:, jtile].T @ qT[:, span]   (psum, causally skipped/sliced)
    p = exp(sT) unshifted (max causal score ~66 fits fp32), bf16; diagonal
        128-blocks multiplied by a 0/1 bf16 triangular mask
    av[0:65, span] += v_nat[jtile].T @ p    (row 64 accumulates sum(p) = denom)
  av -> DRAM; host computes (av[0:64]/av[64]).T per head.

Perf facts measured on this hardware (see also the HAM/tile_position notes):
  - PSUM bank = 512 fp32; matmul outputs stay within one bank
  - keep K=128 and a single 128x128 PE mode everywhere: 64x128 row-tiled pairs
    DO run concurrently but their LDWEIGHTS cannot hide behind same-row-group
    in-flight MMs (~175ns exposed per wall) and the mode mixing throttles the
    HAM clock gate to 1.2 GHz -- measured net LOSS vs plain 128-mode
  - f32r 1.06 cyc/row @2.4GHz warm; ~165ns fixed per MM (~58ns exposed b2b)
  - f32r identity transposes run ~281ns vs ~378ns for fp32 (4-pass)
  - ACT exp = 0.84ns/col + ~250ns/instr and is the attention-phase co-bottleneck:
    span s+1's projection work is interleaved (evenly spread) into span s's
    attention stream at the post-exp insertion point, which is always safe for
    the tile-ring WAR tracking (every live sc tile's reader is already emitted)
  - DVE TensorTensor cannot touch PSUM (BIR verifier); masks ride bf16 SBUF
"""
import sys

if "/opt/trn_rl_repo" not in sys.path:
    sys.path.insert(0, "/opt/trn_rl_repo")

from contextlib import ExitStack

import numpy as np

import concourse.bass as bass
import concourse.tile as tile
from concourse import bacc, mybir, bass_utils
from concourse.masks import make_identity

F32 = mybir.dt.float32
F32R = mybir.dt.float32r
BF16 = mybir.dt.bfloat16

P = 128
H = 64
SPAN = 512
HD = 64

B, N, D, NH = 2, 2048, 768, 12
HL = 3                       # heads per core
DL = HL * HD                 # 192
N_CORES = 8
KT = D // P                  # 6 contraction chunks
KH = KT // 2                 # kt per x/w half
NS = N // SPAN               # 4 spans
NT = N // P                  # 16 j-tiles
CPS = SPAN // P              # 4 chunks per span

DT_PROJ = F32R               # x, W, qT/kT/vT
DT_P = BF16                  # p = exp(scores), v_nat
WARMUP_N = 10                # before first projection
WARMUP_MID = 8               # between kt halves of the first chunk
EXP = mybir.ActivationFunctionType.Exp


def _build(nc, tc, dt_proj, dt_p):
    # host pre-packs: xt[p, (ns, kth, ktl, c)], wc[p, (kt, m)] with
    # m = packed weight columns [q01 | k01 | v01 | k2+v2 | q2]
    xt = nc.dram_tensor("xt", [P, N * KT], dt_proj, kind="ExternalInput").ap()
    wc = nc.dram_tensor("wc", [P, KT * 3 * DL], dt_proj,
                        kind="ExternalInput").ap()
    o = nc.dram_tensor("o", [HL * (HD + 1), N], F32, kind="ExternalOutput").ap()

    with ExitStack() as ctx:
        pool = lambda name, bufs, **kw: ctx.enter_context(
            tc.tile_pool(name=name, bufs=bufs, **kw))
        const_pool = pool("const", 1)
        xpool = pool("x", 2 * NS)
        wpool = pool("w", 2)
        qk_pool = pool("qk", NS)
        kz_pool = pool("kz", HL * NS)
        vnat_pool = pool("vnat", 1)
        ppool = pool("p", 6)
        osb_pool = pool("osb", 3)
        ps = pool("ps", 2, space="PSUM")        # [128,1024] pair tiles: 2x2 banks
        ps_av = pool("ps_av", 4, space="PSUM")  # [65,512] accumulators: 4x1 bank

        ident = const_pool.tile([P, P], F32)
        make_identity(nc, ident[:])
        ident_r = const_pool.tile([P, P], dt_proj)
        nc.vector.tensor_copy(ident_r[:], ident[:])
        # multiplicative causal mask for [key-partition, query-col] diag blocks:
        # 1 where key <= query, 0 where key > query (bf16, post-exp multiply)
        tri16 = const_pool.tile([P, P], dt_p)
        nc.gpsimd.memset(tri16[:], 0.0)
        nc.gpsimd.affine_select(
            out=tri16[:], in_=tri16[:], compare_op=mybir.AluOpType.is_gt,
            fill=1.0, base=0, pattern=[[-1, P]], channel_multiplier=1)
        ones32 = const_pool.tile([P, 2 * NT], F32)
        nc.gpsimd.memset(ones32[:], 1.0)
        zeros = const_pool.tile([P, SPAN], F32)
        nc.gpsimd.memset(zeros[:], 0.0)
        zeros_r = const_pool.tile([P, SPAN], dt_proj)
        nc.vector.tensor_copy(zeros_r[:], zeros[:])

        # ---- DMA inputs: w halves + 8 x chunks (span, kt-half) ----
        x_tiles = [xpool.tile([P, KH * SPAN], dt_proj, tag="x", name=f"x{i}")
                   for i in range(2 * NS)]

        def x_slice(ns, kt):
            t = x_tiles[2 * ns + kt // KH]
            b = (kt % KH) * SPAN
            return t[:, b:b + SPAN]

        def dma_x(ns, half):
            w = KH * SPAN
            i = 2 * ns + half
            nc.sync.dma_start(x_tiles[i][:], xt[:, i * w:(i + 1) * w])

        WCW = 3 * DL
        w_half = [wpool.tile([P, KH * WCW], dt_proj, tag=f"w{i}", name=f"w{i}")
                  for i in range(2)]

        def w_slice(kt, c0, c1):
            t = w_half[kt // KH]
            b = (kt % KH) * WCW
            return t[:, b + c0:b + c1]

        def dma_w(half):
            w = KH * WCW
            nc.sync.dma_start(w_half[half][:], wc[:, half * w:(half + 1) * w])

        dma_w(0)
        dma_x(0, 0)
        dma_x(0, 1)
        dma_w(1)
        for ns in range(1, NS):
            dma_x(ns, 0)
            dma_x(ns, 1)

        # ---- per-span q/k/v tiles (zero-padded K=128 layout) ----
        qT01 = [qk_pool.tile([P, SPAN], dt_proj, tag="q01", name=f"q01_{i}") for i in range(NS)]
        qT2z = [qk_pool.tile([P, SPAN], dt_proj, tag="q2z", name=f"q2z_{i}") for i in range(NS)]
        vT01 = [qk_pool.tile([P, SPAN], dt_proj, tag="v01", name=f"v01_{i}") for i in range(NS)]
        vT2z = [qk_pool.tile([P, SPAN], dt_proj, tag="v2z", name=f"v2z_{i}") for i in range(NS)]
        kTz = [[kz_pool.tile([P, SPAN], dt_proj, tag="kz", name=f"kz_{h}_{i}")
                for i in range(NS)] for h in range(HL)]

        def zfill(ap):
            nc.vector.tensor_copy(ap, zeros[0:ap.shape[0], 0:ap.shape[1]])

        for ns in range(NS):
            zfill(qT2z[ns][HD:P, :])
            zfill(vT2z[ns][0:HD, :])
            zfill(kTz[0][ns][HD:P, :])
            zfill(kTz[1][ns][0:HD, :])
            zfill(kTz[2][ns][HD:P, :])

        # v natural layout: heads 0,1 interleaved per j-tile [v0|1|v1|1], head 2
        # separate [v2|1]; the ones column accumulates the softmax denominator.
        v_nat01 = vnat_pool.tile([P, NT * 2 * (HD + 1)], dt_p, tag="vnat01")
        v_nat2 = vnat_pool.tile([P, NT * (HD + 1)], dt_p, tag="vnat2")
        c01 = v_nat01[:].rearrange("p (t c) -> p t c", c=HD + 1)[:, :, HD]
        c2 = v_nat2[:].rearrange("p (t c) -> p t c", c=HD + 1)[:, :, HD]
        nc.vector.tensor_copy(c01, ones32[:])
        nc.vector.tensor_copy(c2, ones32[:, 0:NT])

        def vnat(h, jt):
            if h < 2:
                b = jt * 2 * (HD + 1) + h * (HD + 1)
                return v_nat01[:, b:b + HD + 1]
            b = jt * (HD + 1)
            return v_nat2[:, b:b + HD + 1]

        # ---- warmup: keep the PE busy while the first DMAs land ----
        warm = ps.tile([P, SPAN], F32, tag="ps", name="warm")

        def warmup(n):
            for _ in range(n):
                nc.tensor.matmul(warm[:], ident_r[:], zeros_r[:],
                                 start=True, stop=True)

        warmup(WARMUP_N)

        # ---- projections as an op list (5 chunk ops + 8 transpose ops) ----
        m_chunks = ((0, P, "q01"), (P, P, "k01"), (2 * P, P, "v01"),
                    (3 * P, P, "k2v2"), (4 * P, HD, "q2"))

        def chunk_op(ns, moff, msz, what, midfill=0):
            pt = ps.tile([msz, SPAN], F32, tag="ps", name=f"pj_{ns}_{what}")
            for kt in range(KT):
                if midfill and kt == KH:
                    warmup(midfill)
                nc.tensor.matmul(
                    pt[:], w_slice(kt, moff, moff + msz), x_slice(ns, kt),
                    start=(kt == 0), stop=(kt == KT - 1))
            if what == "q01":
                nc.vector.tensor_copy(qT01[ns][:], pt[:])
            elif what == "k01":
                nc.vector.tensor_copy(kTz[0][ns][0:HD, :], pt[0:HD, :])
                nc.vector.tensor_copy(kTz[1][ns][HD:P, :], pt[HD:P, :])
            elif what == "v01":
                nc.vector.tensor_copy(vT01[ns][:], pt[:])
            elif what == "k2v2":
                nc.vector.tensor_copy(kTz[2][ns][0:HD, :], pt[0:HD, :])
                nc.vector.tensor_copy(vT2z[ns][HD:P, :], pt[HD:P, :])
            else:
                nc.vector.tensor_copy(qT2z[ns][0:HD, :], pt[:])

        def transp01_op(ns, c):
            jt = ns * CPS + c
            tp = ps.tile([P, P], dt_proj, tag="ps", name=f"tp_{jt}")
            nc.tensor.transpose(tp[:], vT01[ns][:, c * P:(c + 1) * P],
                                ident_r[:])
            nc.vector.tensor_copy(
                v_nat01[:].rearrange("p (t c) -> p t c", c=HD + 1)[
                    :, 2 * jt:2 * jt + 2, 0:HD],
                tp[:].rearrange("p (t c) -> p t c", c=HD))

        def transp2_op(ns, c):
            jt = ns * CPS + c
            tp2 = ps.tile([P, P], dt_proj, tag="ps", name=f"tp2_{jt}")
            nc.tensor.transpose(tp2[:], vT2z[ns][:, c * P:(c + 1) * P],
                                ident_r[:])
            nc.vector.tensor_copy(
                v_nat2[:, jt * (HD + 1):jt * (HD + 1) + HD], tp2[:, HD:P])

        def proj_ops(ns):
            ops = [lambda a=moff, b=msz, w=what: chunk_op(ns, a, b, w)
                   for (moff, msz, what) in m_chunks]
            for c in range(CPS):
                ops.append(lambda c=c: transp01_op(ns, c))
                ops.append(lambda c=c: transp2_op(ns, c))
            return ops

        pending = []
        pavs_left = [1]

        def drain_even():
            # spread pending ops evenly over the remaining insertion points
            if not pending:
                return False
            k = -(-len(pending) // max(pavs_left[0], 1))
            for _ in range(k):
                if pending:
                    pending.pop(0)()
            return True

        def finalize(s, h, av):
            ob = osb_pool.tile([HD + 1, SPAN], F32, tag="osb", name=f"ob{s}_{h}")
            nc.vector.tensor_copy(ob[:], av[:])
            nc.sync.dma_start(
                o[h * (HD + 1):(h + 1) * (HD + 1),
                  s * SPAN:(s + 1) * SPAN], ob[:])

        # ---- attention: heads 0,1 fused pair loop; head 2 solo ----
        def attn01(s):
            njt = CPS * (s + 1)
            av0 = ps_av.tile([HD + 1, SPAN], F32, tag="ps_av", name=f"av0_{s}")
            av1 = ps_av.tile([HD + 1, SPAN], F32, tag="ps_av", name=f"av1_{s}")
            live = {}

            def emit_sc(jt):
                c_d = jt - CPS * s
                n0 = max(c_d, 0) * P
                ns_k, ck = jt // CPS, jt % CPS
                sc = ps.tile([P, 2 * SPAN], F32, tag="ps", name=f"sc01_{s}_{jt}")
                nc.tensor.matmul(sc[:, n0:SPAN],
                                 kTz[0][ns_k][:, ck * P:(ck + 1) * P],
                                 qT01[s][:, n0:SPAN], start=True, stop=True)
                nc.tensor.matmul(sc[:, SPAN + n0:2 * SPAN],
                                 kTz[1][ns_k][:, ck * P:(ck + 1) * P],
                                 qT01[s][:, n0:SPAN], start=True, stop=True)
                live[jt] = (sc, n0, c_d >= 0)

            def emit_pav(jt):
                sc, n0, diag = live.pop(jt)
                p = ppool.tile([P, 2 * SPAN], dt_p, tag="p", name=f"p01_{s}_{jt}")
                sc3 = sc[:].rearrange("q (t c) -> q t c", c=SPAN)
                p3 = p[:].rearrange("q (t c) -> q t c", c=SPAN)
                nc.scalar.activation(p3[:, :, n0:SPAN], sc3[:, :, n0:SPAN], EXP)
                if diag:
                    nc.vector.tensor_mul(p[:, n0:n0 + P], p[:, n0:n0 + P],
                                         tri16[:])
                    nc.vector.tensor_mul(
                        p[:, SPAN + n0:SPAN + n0 + P],
                        p[:, SPAN + n0:SPAN + n0 + P], tri16[:])
                # safe insertion point: every live sc tile's reader is emitted;
                # the proj burst (or the pre-emitted next sc) hides exp latency
                if not drain_even() and jt + 1 < njt and jt + 1 not in live:
                    emit_sc(jt + 1)
                pavs_left[0] -= 1
                st, sp = (jt == 0), (jt == njt - 1)
                nc.tensor.matmul(av0[:, n0:SPAN], vnat(0, jt), p[:, n0:SPAN],
                                 start=st, stop=sp)
                nc.tensor.matmul(av1[:, n0:SPAN], vnat(1, jt),
                                 p[:, SPAN + n0:2 * SPAN], start=st, stop=sp)

            for jt in range(njt):
                if jt not in live:
                    emit_sc(jt)
                emit_pav(jt)
            finalize(s, 0, av0)
            finalize(s, 1, av1)

        def attn2(s):
            njt = CPS * (s + 1)
            av2 = ps_av.tile([HD + 1, SPAN], F32, tag="ps_av", name=f"av2_{s}")
            live = {}

            def emit_sc(jt):
                c_d = jt - CPS * s
                n0 = max(c_d, 0) * P
                ns_k, ck = jt // CPS, jt % CPS
                sc = ps.tile([P, 2 * SPAN], F32, tag="ps", name=f"sc2_{s}_{jt}")
                nc.tensor.matmul(sc[:, n0:SPAN],
                                 kTz[2][ns_k][:, ck * P:(ck + 1) * P],
                                 qT2z[s][:, n0:SPAN], start=True, stop=True)
                live[jt] = (sc, n0, c_d >= 0)

            def emit_pav(jt):
                sc, n0, diag = live.pop(jt)
                p = ppool.tile([P, 2 * SPAN], dt_p, tag="p", name=f"p2_{s}_{jt}")
                nc.scalar.activation(p[:, n0:SPAN], sc[:, n0:SPAN], EXP)
                if diag:
                    nc.vector.tensor_mul(p[:, n0:n0 + P], p[:, n0:n0 + P],
                                         tri16[:])
                if not drain_even() and jt + 1 < njt and jt + 1 not in live:
                    emit_sc(jt + 1)
                pavs_left[0] -= 1
                nc.tensor.matmul(av2[:, n0:SPAN], vnat(2, jt), p[:, n0:SPAN],
                                 start=(jt == 0), stop=(jt == njt - 1))

            for jt in range(njt):
                if jt not in live:
                    emit_sc(jt)
                emit_pav(jt)
            finalize(s, 2, av2)

        # span 0 projections run standalone (warmup mid-fill covers the
        # kt0-2 -> kt3-5 x-DMA boundary of the first chunk); later spans'
        # projections interleave into the previous span's attention stream
        first = True
        for (moff, msz, what) in m_chunks:
            chunk_op(0, moff, msz, what, midfill=WARMUP_MID if first else 0)
            first = False
        for c in range(CPS):
            transp01_op(0, c)
            transp2_op(0, c)
        for s in range(NS):
            pending.extend(proj_ops(s + 1) if s + 1 < NS else [])
            pavs_left[0] = 2 * CPS * (s + 1)
            attn01(s)
            attn2(s)
            while pending:
                pending.pop(0)()


_NC_CACHE = {}


def _get_module(dt_proj=DT_PROJ, dt_p=DT_P):
    key = (dt_proj, dt_p)
    if key not in _NC_CACHE:
        nc = bacc.Bacc("TRN2", target_bir_lowering=False, debug=False)
        with tile.TileContext(nc) as tc:
            _build(nc, tc, dt_proj, dt_p)
        nc.compile()
        _NC_CACHE[key] = nc
    return _NC_CACHE[key]


def _in_maps(x, Wq, Wk, Wv):
    maps = []
    xT = [np.ascontiguousarray(
        x[b].T.reshape(KT, P, NS, SPAN).transpose(1, 2, 0, 3).reshape(P, -1))
        for b in range(B)]
    WqT, WkT, WvT = Wq.T, Wk.T, Wv.T
    for c in range(N_CORES):
        bc, g = divmod(c, N_CORES // B)
        s0 = g * DL
        wcomb = np.concatenate([
            WqT[:, s0:s0 + P], WkT[:, s0:s0 + P], WvT[:, s0:s0 + P],
            WkT[:, s0 + P:s0 + DL], WvT[:, s0 + P:s0 + DL],
            WqT[:, s0 + P:s0 + DL]], axis=1)
        wpk = np.ascontiguousarray(
            wcomb.reshape(KT, P, 3 * DL).transpose(1, 0, 2).reshape(P, -1))
        maps.append({
            "xt": xT[bc],
            "wc": wpk,
        })
    return maps


def kernel(x, Wq, Wk, Wv, _trace=False, _tmpdir=None, **_kw):
    x = np.asarray(x, dtype=np.float32)
    Wq = np.asarray(Wq, dtype=np.float32)
    Wk = np.asarray(Wk, dtype=np.float32)
    Wv = np.asarray(Wv, dtype=np.float32)
    assert x.shape == (B, N, D) and Wq.shape == (D, D)

    nc = _get_module()
    res = bass_utils.run_bass_kernel_spmd(
        nc, _in_maps(x, Wq, Wk, Wv), core_ids=list(range(N_CORES)),
        trace=_trace, tmpdir=_tmpdir)
    out = np.empty((B, N, D), np.float32)
    for c in range(N_CORES):
        bc, g = divmod(c, N_CORES // B)
        oT = res.results[c]["o"].astype(np.float64)
        for h in range(HL):
            blk = oT[h * (HD + 1):h * (HD + 1) + HD, :]
            den = oT[h * (HD + 1) + HD, :]
            out[bc, :, g * DL + h * HD:g * DL + (h + 1) * HD] = \
                (blk / den).T.astype(np.float32)
    if _trace:
        return out, res
    return out


# revision 30
# speedup vs baseline: 1.2055x; 1.1812x over previous
"""Causal multi-head attention (b=2, n=2048, d=768, 12 heads) on 8 TRN2 NeuronCores.

Sharding: batch x head-group. Core c handles batch c//4 and heads 3*(c%4) .. 3*(c%4)+2.
Each core gets xT = x[b].T plus W.T column slices for its 3 heads, computes the
unnormalized attention output (transposed) plus softmax denominators; the host
divides, transposes, and concatenates slabs into the full [2, 2048, 768].

Per-core algorithm (everything transposed so softmax reductions ride on matmuls):
  qT/kT/vT = (W.T slice).T @ xT            TensorE, per 512-col span
  v_nat[j, m] = transpose(vT) + ones column -> stationary [128, 65] per j-tile
  per head, per 512-col i-span:
    sT[j, i] = kT_h[:, jtile].T @ qT[:, span]   (psum, causally skipped/sliced)
    p = exp(sT) unshifted (max causal score ~66 fits fp32), bf16; diagonal
        128-blocks multiplied by a 0/1 bf16 triangular mask
    av[0:65, span] += v_nat[jtile].T @ p    (row 64 accumulates sum(p) = denom)
  av -> DRAM; host computes (av[0:64]/av[64]).T per head.

Perf facts measured on this hardware (see also the HAM/tile_position notes):
  - PSUM bank = 512 fp32; matmul outputs stay within one bank
  - keep K=128 and a single 128x128 PE mode everywhere: 64x128 row-tiled pairs
    DO run concurrently but their LDWEIGHTS cannot hide behind same-row-group
    in-flight MMs (~175ns exposed per wall) and the mode mixing throttles the
    HAM clock gate to 1.2 GHz -- measured net LOSS vs plain 128-mode
  - f32r 1.06 cyc/row @2.4GHz warm; ~165ns fixed per MM (~58ns exposed b2b)
  - f32r identity transposes run ~281ns vs ~378ns for fp32 (4-pass)
  - ACT exp = 0.84ns/col + ~250ns/instr and is the attention-phase co-bottleneck:
    span s+1's projection work is interleaved (evenly spread) into span s's
    attention stream at the post-exp insertion point, which is always safe for
    the tile-ring WAR tracking (every live sc tile's reader is already emitted)
  - DVE TensorTensor cannot touch PSUM (BIR verifier); masks ride bf16 SBUF
"""
import sys

if "/opt/trn_rl_repo" not in sys.path:
    sys.path.insert(0, "/opt/trn_rl_repo")

from contextlib import ExitStack

import numpy as np

import concourse.bass as bass
import concourse.tile as tile
from concourse import bacc, mybir, bass_utils
from concourse.masks import make_identity

F32 = mybir.dt.float32
F32R = mybir.dt.float32r
BF16 = mybir.dt.bfloat16

P = 128
H = 64
SPAN = 512
HD = 64

B, N, D, NH = 2, 2048, 768, 12
HL = 3                       # heads per core
DL = HL * HD                 # 192
N_CORES = 8
KT = D // P                  # 6 contraction chunks
KH = KT // 2                 # kt per x/w half
NS = N // SPAN               # 4 spans
NT = N // P                  # 16 j-tiles
CPS = SPAN // P              # 4 chunks per span

DT_PROJ = F32R               # x, W, qT/kT/vT
DT_P = BF16                  # p = exp(scores), v_nat
WARMUP_N = 10                # before first projection
WARMUP_MID = 8               # between kt halves of the first chunk
EXP = mybir.ActivationFunctionType.Exp


def _build(nc, tc, dt_proj, dt_p):
    # host pre-packs: xt[p, (ns, kth, ktl, c)], wc[p, (kt, m)] with
    # m = packed weight columns [q01 | k01 | v01 | k2+v2 | q2]
    xt = nc.dram_tensor("xt", [P, N * KT], dt_proj, kind="ExternalInput").ap()
    wc = nc.dram_tensor("wc", [P, KT * 3 * DL], dt_proj,
                        kind="ExternalInput").ap()
    o = nc.dram_tensor("o", [HL * (HD + 1), N], F32, kind="ExternalOutput").ap()

    with ExitStack() as ctx:
        pool = lambda name, bufs, **kw: ctx.enter_context(
            tc.tile_pool(name=name, bufs=bufs, **kw))
        const_pool = pool("const", 1)
        xpool = pool("x", 2 * NS)
        wpool = pool("w", 2)
        qk_pool = pool("qk", NS)
        kz_pool = pool("kz", HL * NS)
        vnat_pool = pool("vnat", 1)
        ppool = pool("p", 6)
        osb_pool = pool("osb", 3)
        ps = pool("ps", 2, space="PSUM")        # [128,1024] sc pair tiles: 2x2 banks
        ps_pj = pool("ps_pj", 2, space="PSUM")  # [128,512] proj/transpose: 2x1 bank
        ps_av = pool("ps_av", 2, space="PSUM")  # [65,512] accumulators: 2x1 bank

        ident = const_pool.tile([P, P], F32)
        make_identity(nc, ident[:])
        ident_r = const_pool.tile([P, P], dt_proj)
        nc.vector.tensor_copy(ident_r[:], ident[:])
        # multiplicative causal mask for [key-partition, query-col] diag blocks:
        # 1 where key <= query, 0 where key > query (bf16, post-exp multiply)
        tri16 = const_pool.tile([P, P], dt_p)
        nc.gpsimd.memset(tri16[:], 0.0)
        nc.gpsimd.affine_select(
            out=tri16[:], in_=tri16[:], compare_op=mybir.AluOpType.is_gt,
            fill=1.0, base=0, pattern=[[-1, P]], channel_multiplier=1)
        ones32 = const_pool.tile([P, 2 * NT], F32)
        nc.gpsimd.memset(ones32[:], 1.0)
        zeros = const_pool.tile([P, SPAN], F32)
        nc.gpsimd.memset(zeros[:], 0.0)
        zeros_r = const_pool.tile([P, SPAN], dt_proj)
        nc.vector.tensor_copy(zeros_r[:], zeros[:])

        # ---- DMA inputs: w halves + 8 x chunks (span, kt-half) ----
        x_tiles = [xpool.tile([P, KH * SPAN], dt_proj, tag="x", name=f"x{i}")
                   for i in range(2 * NS)]

        def x_slice(ns, kt):
            t = x_tiles[2 * ns + kt // KH]
            b = (kt % KH) * SPAN
            return t[:, b:b + SPAN]

        def dma_x(ns, half):
            w = KH * SPAN
            i = 2 * ns + half
            nc.sync.dma_start(x_tiles[i][:], xt[:, i * w:(i + 1) * w])

        WCW = 3 * DL
        w_half = [wpool.tile([P, KH * WCW], dt_proj, tag=f"w{i}", name=f"w{i}")
                  for i in range(2)]

        def w_slice(kt, c0, c1):
            t = w_half[kt // KH]
            b = (kt % KH) * WCW
            return t[:, b + c0:b + c1]

        def dma_w(half):
            w = KH * WCW
            nc.sync.dma_start(w_half[half][:], wc[:, half * w:(half + 1) * w])

        dma_w(0)
        dma_x(0, 0)
        dma_x(0, 1)
        dma_w(1)
        for ns in range(1, NS):
            dma_x(ns, 0)
            dma_x(ns, 1)

        # ---- per-span q/k/v tiles (zero-padded K=128 layout) ----
        qT01 = [qk_pool.tile([P, SPAN], dt_proj, tag="q01", name=f"q01_{i}") for i in range(NS)]
        qT2z = [qk_pool.tile([P, SPAN], dt_proj, tag="q2z", name=f"q2z_{i}") for i in range(NS)]
        vT01 = [qk_pool.tile([P, SPAN], dt_proj, tag="v01", name=f"v01_{i}") for i in range(NS)]
        vT2z = [qk_pool.tile([P, SPAN], dt_proj, tag="v2z", name=f"v2z_{i}") for i in range(NS)]
        kTz = [[kz_pool.tile([P, SPAN], dt_proj, tag="kz", name=f"kz_{h}_{i}")
                for i in range(NS)] for h in range(HL)]

        def zfill(ap):
            nc.vector.tensor_copy(ap, zeros[0:ap.shape[0], 0:ap.shape[1]])

        for ns in range(NS):
            zfill(qT2z[ns][HD:P, :])
            zfill(vT2z[ns][0:HD, :])
            zfill(kTz[0][ns][HD:P, :])
            zfill(kTz[1][ns][0:HD, :])
            zfill(kTz[2][ns][HD:P, :])

        # v natural layout: heads 0,1 interleaved per j-tile [v0|1|v1|1], head 2
        # separate [v2|1]; the ones column accumulates the softmax denominator.
        v_nat01 = vnat_pool.tile([P, NT * 2 * (HD + 1)], dt_p, tag="vnat01")
        v_nat2 = vnat_pool.tile([P, NT * (HD + 1)], dt_p, tag="vnat2")
        c01 = v_nat01[:].rearrange("p (t c) -> p t c", c=HD + 1)[:, :, HD]
        c2 = v_nat2[:].rearrange("p (t c) -> p t c", c=HD + 1)[:, :, HD]
        nc.vector.tensor_copy(c01, ones32[:])
        nc.vector.tensor_copy(c2, ones32[:, 0:NT])

        def vnat(h, jt):
            if h < 2:
                b = jt * 2 * (HD + 1) + h * (HD + 1)
                return v_nat01[:, b:b + HD + 1]
            b = jt * (HD + 1)
            return v_nat2[:, b:b + HD + 1]

        # ---- warmup: keep the PE busy while the first DMAs land ----
        warm = ps_pj.tile([P, SPAN], F32, tag="ps_pj", name="warm")

        def warmup(n):
            for _ in range(n):
                nc.tensor.matmul(warm[:], ident_r[:], zeros_r[:],
                                 start=True, stop=True)

        warmup(WARMUP_N)

        # ---- projections as an op list (5 chunk ops + 8 transpose ops) ----
        m_chunks = ((0, P, "q01"), (P, P, "k01"), (2 * P, P, "v01"),
                    (3 * P, P, "k2v2"), (4 * P, HD, "q2"))

        def chunk_op(ns, moff, msz, what, midfill=0):
            pt = ps_pj.tile([msz, SPAN], F32, tag="ps_pj", name=f"pj_{ns}_{what}")
            for kt in range(KT):
                if midfill and kt == KH:
                    warmup(midfill)
                nc.tensor.matmul(
                    pt[:], w_slice(kt, moff, moff + msz), x_slice(ns, kt),
                    start=(kt == 0), stop=(kt == KT - 1))
            if what == "q01":
                nc.vector.tensor_copy(qT01[ns][:], pt[:])
            elif what == "k01":
                nc.vector.tensor_copy(kTz[0][ns][0:HD, :], pt[0:HD, :])
                nc.vector.tensor_copy(kTz[1][ns][HD:P, :], pt[HD:P, :])
            elif what == "v01":
                nc.vector.tensor_copy(vT01[ns][:], pt[:])
            elif what == "k2v2":
                nc.vector.tensor_copy(kTz[2][ns][0:HD, :], pt[0:HD, :])
                nc.vector.tensor_copy(vT2z[ns][HD:P, :], pt[HD:P, :])
            else:
                nc.vector.tensor_copy(qT2z[ns][0:HD, :], pt[:])

        def transp01_op(ns, c):
            jt = ns * CPS + c
            tp = ps_pj.tile([P, P], dt_proj, tag="ps_pj", name=f"tp_{jt}")
            nc.tensor.transpose(tp[:], vT01[ns][:, c * P:(c + 1) * P],
                                ident_r[:])
            nc.vector.tensor_copy(
                v_nat01[:].rearrange("p (t c) -> p t c", c=HD + 1)[
                    :, 2 * jt:2 * jt + 2, 0:HD],
                tp[:].rearrange("p (t c) -> p t c", c=HD))

        def transp2_op(ns, c):
            jt = ns * CPS + c
            tp2 = ps_pj.tile([P, P], dt_proj, tag="ps_pj", name=f"tp2_{jt}")
            nc.tensor.transpose(tp2[:], vT2z[ns][:, c * P:(c + 1) * P],
                                ident_r[:])
            nc.vector.tensor_copy(
                v_nat2[:, jt * (HD + 1):jt * (HD + 1) + HD], tp2[:, HD:P])

        def proj_ops(ns):
            ops = [lambda a=moff, b=msz, w=what: chunk_op(ns, a, b, w)
                   for (moff, msz, what) in m_chunks]
            for c in range(CPS):
                ops.append(lambda c=c: transp01_op(ns, c))
                ops.append(lambda c=c: transp2_op(ns, c))
            return ops

        pending = []
        pavs_left = [1]

        def drain_even():
            # spread pending ops evenly over the remaining insertion points
            if not pending:
                return False
            k = -(-len(pending) // max(pavs_left[0], 1))
            for _ in range(k):
                if pending:
                    pending.pop(0)()
            return True

        def finalize(s, h, av):
            ob = osb_pool.tile([HD + 1, SPAN], F32, tag="osb", name=f"ob{s}_{h}")
            nc.vector.tensor_copy(ob[:], av[:])
            nc.sync.dma_start(
                o[h * (HD + 1):(h + 1) * (HD + 1),
                  s * SPAN:(s + 1) * SPAN], ob[:])

        # ---- attention: heads 0,1 fused pair loop; head 2 solo ----
        def attn01(s):
            njt = CPS * (s + 1)
            av0 = ps_av.tile([HD + 1, SPAN], F32, tag="ps_av", name=f"av0_{s}")
            av1 = ps_av.tile([HD + 1, SPAN], F32, tag="ps_av", name=f"av1_{s}")
            live = {}

            def emit_sc(jt):
                c_d = jt - CPS * s
                n0 = max(c_d, 0) * P
                ns_k, ck = jt // CPS, jt % CPS
                sc = ps.tile([P, 2 * SPAN], F32, tag="ps", name=f"sc01_{s}_{jt}")
                nc.tensor.matmul(sc[:, n0:SPAN],
                                 kTz[0][ns_k][:, ck * P:(ck + 1) * P],
                                 qT01[s][:, n0:SPAN], start=True, stop=True)
                nc.tensor.matmul(sc[:, SPAN + n0:2 * SPAN],
                                 kTz[1][ns_k][:, ck * P:(ck + 1) * P],
                                 qT01[s][:, n0:SPAN], start=True, stop=True)
                live[jt] = (sc, n0, c_d >= 0)

            def emit_pav(jt):
                sc, n0, diag = live.pop(jt)
                p = ppool.tile([P, 2 * SPAN], dt_p, tag="p", name=f"p01_{s}_{jt}")
                sc3 = sc[:].rearrange("q (t c) -> q t c", c=SPAN)
                p3 = p[:].rearrange("q (t c) -> q t c", c=SPAN)
                nc.scalar.activation(p3[:, :, n0:SPAN], sc3[:, :, n0:SPAN], EXP)
                if diag:
                    nc.vector.tensor_mul(p[:, n0:n0 + P], p[:, n0:n0 + P],
                                         tri16[:])
                    nc.vector.tensor_mul(
                        p[:, SPAN + n0:SPAN + n0 + P],
                        p[:, SPAN + n0:SPAN + n0 + P], tri16[:])
                # safe insertion point: every live sc tile's reader is emitted;
                # the proj burst (or the pre-emitted next sc) hides exp latency
                if not drain_even() and jt + 1 < njt and jt + 1 not in live:
                    emit_sc(jt + 1)
                pavs_left[0] -= 1
                st, sp = (jt == 0), (jt == njt - 1)
                nc.tensor.matmul(av0[:, n0:SPAN], vnat(0, jt), p[:, n0:SPAN],
                                 start=st, stop=sp)
                nc.tensor.matmul(av1[:, n0:SPAN], vnat(1, jt),
                                 p[:, SPAN + n0:2 * SPAN], start=st, stop=sp)

            for jt in range(njt):
                if jt not in live:
                    emit_sc(jt)
                emit_pav(jt)
            finalize(s, 0, av0)
            finalize(s, 1, av1)

        def attn2(s):
            njt = CPS * (s + 1)
            av2 = ps_av.tile([HD + 1, SPAN], F32, tag="ps_av", name=f"av2_{s}")
            live = {}

            def emit_sc(jt):
                c_d = jt - CPS * s
                n0 = max(c_d, 0) * P
                ns_k, ck = jt // CPS, jt % CPS
                sc = ps.tile([P, 2 * SPAN], F32, tag="ps", name=f"sc2_{s}_{jt}")
                nc.tensor.matmul(sc[:, n0:SPAN],
                                 kTz[2][ns_k][:, ck * P:(ck + 1) * P],
                                 qT2z[s][:, n0:SPAN], start=True, stop=True)
                live[jt] = (sc, n0, c_d >= 0)

            def emit_pav(jt):
                sc, n0, diag = live.pop(jt)
                p = ppool.tile([P, 2 * SPAN], dt_p, tag="p", name=f"p2_{s}_{jt}")
                nc.scalar.activation(p[:, n0:SPAN], sc[:, n0:SPAN], EXP)
                if diag:
                    nc.vector.tensor_mul(p[:, n0:n0 + P], p[:, n0:n0 + P],
                                         tri16[:])
                if not drain_even() and jt + 1 < njt and jt + 1 not in live:
                    emit_sc(jt + 1)
                pavs_left[0] -= 1
                nc.tensor.matmul(av2[:, n0:SPAN], vnat(2, jt), p[:, n0:SPAN],
                                 start=(jt == 0), stop=(jt == njt - 1))

            for jt in range(njt):
                if jt not in live:
                    emit_sc(jt)
                emit_pav(jt)
            finalize(s, 2, av2)

        # span 0 projections run standalone (warmup mid-fill covers the
        # kt0-2 -> kt3-5 x-DMA boundary of the first chunk); later spans'
        # projections interleave into the previous span's attention stream
        first = True
        for (moff, msz, what) in m_chunks:
            chunk_op(0, moff, msz, what, midfill=WARMUP_MID if first else 0)
            first = False
        for c in range(CPS):
            transp01_op(0, c)
            transp2_op(0, c)
        for s in range(NS):
            pending.extend(proj_ops(s + 1) if s + 1 < NS else [])
            pavs_left[0] = 2 * CPS * (s + 1)
            attn01(s)
            attn2(s)
            while pending:
                pending.pop(0)()


_NC_CACHE = {}


def _get_module(dt_proj=DT_PROJ, dt_p=DT_P):
    key = (dt_proj, dt_p)
    if key not in _NC_CACHE:
        nc = bacc.Bacc("TRN2", target_bir_lowering=False, debug=False)
        with tile.TileContext(nc) as tc:
            _build(nc, tc, dt_proj, dt_p)
        nc.compile()
        _NC_CACHE[key] = nc
    return _NC_CACHE[key]


def _in_maps(x, Wq, Wk, Wv):
    maps = []
    xT = [np.ascontiguousarray(
        x[b].T.reshape(KT, P, NS, SPAN).transpose(1, 2, 0, 3).reshape(P, -1))
        for b in range(B)]
    WqT, WkT, WvT = Wq.T, Wk.T, Wv.T
    for c in range(N_CORES):
        bc, g = divmod(c, N_CORES // B)
        s0 = g * DL
        wcomb = np.concatenate([
            WqT[:, s0:s0 + P], WkT[:, s0:s0 + P], WvT[:, s0:s0 + P],
            WkT[:, s0 + P:s0 + DL], WvT[:, s0 + P:s0 + DL],
            WqT[:, s0 + P:s0 + DL]], axis=1)
        wpk = np.ascontiguousarray(
            wcomb.reshape(KT, P, 3 * DL).transpose(1, 0, 2).reshape(P, -1))
        maps.append({
            "xt": xT[bc],
            "wc": wpk,
        })
    return maps


def kernel(x, Wq, Wk, Wv, _trace=False, _tmpdir=None, **_kw):
    x = np.asarray(x, dtype=np.float32)
    Wq = np.asarray(Wq, dtype=np.float32)
    Wk = np.asarray(Wk, dtype=np.float32)
    Wv = np.asarray(Wv, dtype=np.float32)
    assert x.shape == (B, N, D) and Wq.shape == (D, D)

    nc = _get_module()
    res = bass_utils.run_bass_kernel_spmd(
        nc, _in_maps(x, Wq, Wk, Wv), core_ids=list(range(N_CORES)),
        trace=_trace, tmpdir=_tmpdir)
    out = np.empty((B, N, D), np.float32)
    for c in range(N_CORES):
        bc, g = divmod(c, N_CORES // B)
        oT = res.results[c]["o"].astype(np.float64)
        for h in range(HL):
            blk = oT[h * (HD + 1):h * (HD + 1) + HD, :]
            den = oT[h * (HD + 1) + HD, :]
            out[bc, :, g * DL + h * HD:g * DL + (h + 1) * HD] = \
                (blk / den).T.astype(np.float32)
    if _trace:
        return out, res
    return out


# revision 34
# speedup vs baseline: 1.2317x; 1.0218x over previous
"""Causal multi-head attention (b=2, n=2048, d=768, 12 heads) on 8 TRN2 NeuronCores.

Sharding: batch x head-group. Core c handles batch c//4 and heads 3*(c%4) .. 3*(c%4)+2.
Each core gets xT = x[b].T plus W.T column slices for its 3 heads, computes the
unnormalized attention output (transposed) plus softmax denominators; the host
divides, transposes, and concatenates slabs into the full [2, 2048, 768].

Per-core algorithm (everything transposed so softmax reductions ride on matmuls):
  qT/kT/vT = (W.T slice).T @ xT            TensorE, per 512-col span
  v_nat[j, m] = transpose(vT) + ones column -> stationary [128, 65] per j-tile
  per head, per 512-col i-span:
    sT[j, i] = kT_h[:, jtile].T @ qT[:, span]   (psum, causally skipped/sliced)
    p = exp(sT) unshifted (max causal score ~66 fits fp32), bf16; diagonal
        128-blocks multiplied by a 0/1 bf16 triangular mask
    av[0:65, span] += v_nat[jtile].T @ p    (row 64 accumulates sum(p) = denom)
  av -> DRAM; host computes (av[0:64]/av[64]).T per head.

Perf facts measured on this hardware (see also the HAM/tile_position notes):
  - PSUM bank = 512 fp32; matmul outputs stay within one bank
  - keep K=128 and a single 128x128 PE mode everywhere: 64x128 row-tiled pairs
    DO run concurrently but their LDWEIGHTS cannot hide behind same-row-group
    in-flight MMs (~175ns exposed per wall) and the mode mixing throttles the
    HAM clock gate to 1.2 GHz -- measured net LOSS vs plain 128-mode
  - f32r 1.06 cyc/row @2.4GHz warm; ~165ns fixed per MM (~58ns exposed b2b)
  - f32r identity transposes run ~281ns vs ~378ns for fp32 (4-pass)
  - ACT exp = 0.84ns/col + ~250ns/instr and is the attention-phase co-bottleneck:
    span s+1's projection work is interleaved (evenly spread) into span s's
    attention stream at the post-exp insertion point, which is always safe for
    the tile-ring WAR tracking (every live sc tile's reader is already emitted)
  - DVE TensorTensor cannot touch PSUM (BIR verifier); masks ride bf16 SBUF
"""
import sys

if "/opt/trn_rl_repo" not in sys.path:
    sys.path.insert(0, "/opt/trn_rl_repo")

from contextlib import ExitStack

import numpy as np

import concourse.bass as bass
import concourse.tile as tile
from concourse import bacc, mybir, bass_utils
from concourse.masks import make_identity

F32 = mybir.dt.float32
F32R = mybir.dt.float32r
BF16 = mybir.dt.bfloat16

P = 128
H = 64
SPAN = 512
HD = 64

B, N, D, NH = 2, 2048, 768, 12
HL = 3                       # heads per core
DL = HL * HD                 # 192
N_CORES = 8
KT = D // P                  # 6 contraction chunks
KH = KT // 2                 # kt per x/w half
NS = N // SPAN               # 4 spans
NT = N // P                  # 16 j-tiles
CPS = SPAN // P              # 4 chunks per span

DT_PROJ = F32R               # x, W, qT/kT/vT
DT_P = BF16                  # p = exp(scores), v_nat
WARMUP_N = 12                # before first projection
WARMUP_MID = 10              # between kt halves of the first chunk
EXP = mybir.ActivationFunctionType.Exp


def _build(nc, tc, dt_proj, dt_p):
    # host pre-packs: xt[p, (ns, kth, ktl, c)], wc[p, (kt, m)] with
    # m = packed weight columns [q01 | k01 | v01 | k2+v2 | q2]
    xt = nc.dram_tensor("xt", [P, N * KT], dt_proj, kind="ExternalInput").ap()
    wc = nc.dram_tensor("wc", [P, KT * 3 * DL], dt_proj,
                        kind="ExternalInput").ap()
    o = nc.dram_tensor("o", [HL * (HD + 1), N], F32, kind="ExternalOutput").ap()

    with ExitStack() as ctx:
        pool = lambda name, bufs, **kw: ctx.enter_context(
            tc.tile_pool(name=name, bufs=bufs, **kw))
        const_pool = pool("const", 1)
        xpool = pool("x", 2 * NS)
        wpool = pool("w", 2)
        qk_pool = pool("qk", NS)
        kz_pool = pool("kz", HL * NS)
        vnat_pool = pool("vnat", 1)
        ppool = pool("p", 6)
        osb_pool = pool("osb", 3)
        ps = pool("ps", 2, space="PSUM")        # [128,1024] sc pair tiles: 2x2 banks
        ps_pj = pool("ps_pj", 2, space="PSUM")  # [128,512] proj/transpose: 2x1 bank
        ps_av = pool("ps_av", 2, space="PSUM")  # [65,512] accumulators: 2x1 bank

        ident = const_pool.tile([P, P], F32)
        make_identity(nc, ident[:])
        ident_r = const_pool.tile([P, P], dt_proj)
        nc.vector.tensor_copy(ident_r[:], ident[:])
        # multiplicative causal mask for [key-partition, query-col] diag blocks:
        # 1 where key <= query, 0 where key > query (bf16, post-exp multiply)
        tri16 = const_pool.tile([P, P], dt_p)
        nc.gpsimd.memset(tri16[:], 0.0)
        nc.gpsimd.affine_select(
            out=tri16[:], in_=tri16[:], compare_op=mybir.AluOpType.is_gt,
            fill=1.0, base=0, pattern=[[-1, P]], channel_multiplier=1)
        ones32 = const_pool.tile([P, 2 * NT], F32)
        nc.gpsimd.memset(ones32[:], 1.0)
        zeros = const_pool.tile([P, SPAN], F32)
        nc.gpsimd.memset(zeros[:], 0.0)
        zeros_r = const_pool.tile([P, SPAN], dt_proj)
        nc.vector.tensor_copy(zeros_r[:], zeros[:])

        # ---- DMA inputs: w halves + 8 x chunks (span, kt-half) ----
        x_tiles = [xpool.tile([P, KH * SPAN], dt_proj, tag="x", name=f"x{i}")
                   for i in range(2 * NS)]

        def x_slice(ns, kt):
            t = x_tiles[2 * ns + kt // KH]
            b = (kt % KH) * SPAN
            return t[:, b:b + SPAN]

        def dma_x(ns, half):
            w = KH * SPAN
            i = 2 * ns + half
            nc.sync.dma_start(x_tiles[i][:], xt[:, i * w:(i + 1) * w])

        WCW = 3 * DL
        w_half = [wpool.tile([P, KH * WCW], dt_proj, tag=f"w{i}", name=f"w{i}")
                  for i in range(2)]

        def w_slice(kt, c0, c1):
            t = w_half[kt // KH]
            b = (kt % KH) * WCW
            return t[:, b + c0:b + c1]

        def dma_w(half):
            w = KH * WCW
            nc.sync.dma_start(w_half[half][:], wc[:, half * w:(half + 1) * w])

        dma_w(0)
        dma_x(0, 0)
        dma_x(0, 1)
        dma_w(1)
        for ns in range(1, NS):
            dma_x(ns, 0)
            dma_x(ns, 1)

        # ---- per-span q/k/v tiles (zero-padded K=128 layout) ----
        qT01 = [qk_pool.tile([P, SPAN], dt_proj, tag="q01", name=f"q01_{i}") for i in range(NS)]
        qT2z = [qk_pool.tile([P, SPAN], dt_proj, tag="q2z", name=f"q2z_{i}") for i in range(NS)]
        vT01 = [qk_pool.tile([P, SPAN], dt_proj, tag="v01", name=f"v01_{i}") for i in range(NS)]
        vT2z = [qk_pool.tile([P, SPAN], dt_proj, tag="v2z", name=f"v2z_{i}") for i in range(NS)]
        kTz = [[kz_pool.tile([P, SPAN], dt_proj, tag="kz", name=f"kz_{h}_{i}")
                for i in range(NS)] for h in range(HL)]

        def zfill(ap):
            nc.vector.tensor_copy(ap, zeros[0:ap.shape[0], 0:ap.shape[1]])

        for ns in range(NS):
            zfill(qT2z[ns][HD:P, :])
            zfill(vT2z[ns][0:HD, :])
            zfill(kTz[0][ns][HD:P, :])
            zfill(kTz[1][ns][0:HD, :])
            zfill(kTz[2][ns][HD:P, :])

        # v natural layout: heads 0,1 interleaved per j-tile [v0|1|v1|1], head 2
        # separate [v2|1]; the ones column accumulates the softmax denominator.
        v_nat01 = vnat_pool.tile([P, NT * 2 * (HD + 1)], dt_p, tag="vnat01")
        v_nat2 = vnat_pool.tile([P, NT * (HD + 1)], dt_p, tag="vnat2")
        c01 = v_nat01[:].rearrange("p (t c) -> p t c", c=HD + 1)[:, :, HD]
        c2 = v_nat2[:].rearrange("p (t c) -> p t c", c=HD + 1)[:, :, HD]
        nc.vector.tensor_copy(c01, ones32[:])
        nc.vector.tensor_copy(c2, ones32[:, 0:NT])

        def vnat(h, jt):
            if h < 2:
                b = jt * 2 * (HD + 1) + h * (HD + 1)
                return v_nat01[:, b:b + HD + 1]
            b = jt * (HD + 1)
            return v_nat2[:, b:b + HD + 1]

        # ---- warmup: keep the PE busy while the first DMAs land ----
        warm = ps_pj.tile([P, SPAN], F32, tag="ps_pj", name="warm")

        def warmup(n):
            for _ in range(n):
                nc.tensor.matmul(warm[:], ident_r[:], zeros_r[:],
                                 start=True, stop=True)

        warmup(WARMUP_N)

        # ---- projections as an op list (5 chunk ops + 8 transpose ops) ----
        m_chunks = ((0, P, "q01"), (P, P, "k01"), (2 * P, P, "v01"),
                    (3 * P, P, "k2v2"), (4 * P, HD, "q2"))

        def chunk_op(ns, moff, msz, what, midfill=0):
            pt = ps_pj.tile([msz, SPAN], F32, tag="ps_pj", name=f"pj_{ns}_{what}")
            for kt in range(KT):
                if midfill and kt == KH:
                    warmup(midfill)
                nc.tensor.matmul(
                    pt[:], w_slice(kt, moff, moff + msz), x_slice(ns, kt),
                    start=(kt == 0), stop=(kt == KT - 1))
            if what == "q01":
                nc.vector.tensor_copy(qT01[ns][:], pt[:])
            elif what == "k01":
                nc.vector.tensor_copy(kTz[0][ns][0:HD, :], pt[0:HD, :])
                nc.vector.tensor_copy(kTz[1][ns][HD:P, :], pt[HD:P, :])
            elif what == "v01":
                nc.vector.tensor_copy(vT01[ns][:], pt[:])
            elif what == "k2v2":
                nc.vector.tensor_copy(kTz[2][ns][0:HD, :], pt[0:HD, :])
                nc.vector.tensor_copy(vT2z[ns][HD:P, :], pt[HD:P, :])
            else:
                nc.vector.tensor_copy(qT2z[ns][0:HD, :], pt[:])

        def transp01_op(ns, c):
            jt = ns * CPS + c
            tp = ps_pj.tile([P, P], dt_proj, tag="ps_pj", name=f"tp_{jt}")
            nc.tensor.transpose(tp[:], vT01[ns][:, c * P:(c + 1) * P],
                                ident_r[:])
            nc.vector.tensor_copy(
                v_nat01[:].rearrange("p (t c) -> p t c", c=HD + 1)[
                    :, 2 * jt:2 * jt + 2, 0:HD],
                tp[:].rearrange("p (t c) -> p t c", c=HD))

        def transp2_op(ns, c):
            jt = ns * CPS + c
            tp2 = ps_pj.tile([P, P], dt_proj, tag="ps_pj", name=f"tp2_{jt}")
            nc.tensor.transpose(tp2[:], vT2z[ns][:, c * P:(c + 1) * P],
                                ident_r[:])
            nc.vector.tensor_copy(
                v_nat2[:, jt * (HD + 1):jt * (HD + 1) + HD], tp2[:, HD:P])

        def proj_ops(ns):
            ops = [lambda a=moff, b=msz, w=what: chunk_op(ns, a, b, w)
                   for (moff, msz, what) in m_chunks]
            for c in range(CPS):
                ops.append(lambda c=c: transp01_op(ns, c))
                ops.append(lambda c=c: transp2_op(ns, c))
            return ops

        pending = []
        pavs_left = [1]

        def drain_even():
            # spread pending ops evenly over the remaining insertion points
            if not pending:
                return False
            k = -(-len(pending) // max(pavs_left[0], 1))
            for _ in range(k):
                if pending:
                    pending.pop(0)()
            return True

        def finalize(s, h, av):
            ob = osb_pool.tile([HD + 1, SPAN], F32, tag="osb", name=f"ob{s}_{h}")
            nc.vector.tensor_copy(ob[:], av[:])
            nc.sync.dma_start(
                o[h * (HD + 1):(h + 1) * (HD + 1),
                  s * SPAN:(s + 1) * SPAN], ob[:])

        # ---- attention: heads 0,1 fused pair loop; head 2 solo ----
        def attn01(s):
            njt = CPS * (s + 1)
            av0 = ps_av.tile([HD + 1, SPAN], F32, tag="ps_av", name=f"av0_{s}")
            av1 = ps_av.tile([HD + 1, SPAN], F32, tag="ps_av", name=f"av1_{s}")
            live = {}

            def emit_sc(jt):
                c_d = jt - CPS * s
                n0 = max(c_d, 0) * P
                ns_k, ck = jt // CPS, jt % CPS
                sc = ps.tile([P, 2 * SPAN], F32, tag="ps", name=f"sc01_{s}_{jt}")
                nc.tensor.matmul(sc[:, n0:SPAN],
                                 kTz[0][ns_k][:, ck * P:(ck + 1) * P],
                                 qT01[s][:, n0:SPAN], start=True, stop=True)
                nc.tensor.matmul(sc[:, SPAN + n0:2 * SPAN],
                                 kTz[1][ns_k][:, ck * P:(ck + 1) * P],
                                 qT01[s][:, n0:SPAN], start=True, stop=True)
                live[jt] = (sc, n0, c_d >= 0)

            def emit_pav(jt):
                sc, n0, diag = live.pop(jt)
                p = ppool.tile([P, 2 * SPAN], dt_p, tag="p", name=f"p01_{s}_{jt}")
                sc3 = sc[:].rearrange("q (t c) -> q t c", c=SPAN)
                p3 = p[:].rearrange("q (t c) -> q t c", c=SPAN)
                nc.scalar.activation(p3[:, :, n0:SPAN], sc3[:, :, n0:SPAN], EXP)
                if diag:
                    nc.vector.tensor_mul(p[:, n0:n0 + P], p[:, n0:n0 + P],
                                         tri16[:])
                    nc.vector.tensor_mul(
                        p[:, SPAN + n0:SPAN + n0 + P],
                        p[:, SPAN + n0:SPAN + n0 + P], tri16[:])
                # safe insertion point: every live sc tile's reader is emitted;
                # proj bursts and the two-ahead sc keep Tensor fed through the
                # exp latency
                drain_even()
                if jt + 2 < njt and jt + 2 not in live:
                    emit_sc(jt + 2)
                pavs_left[0] -= 1
                st, sp = (jt == 0), (jt == njt - 1)
                nc.tensor.matmul(av0[:, n0:SPAN], vnat(0, jt), p[:, n0:SPAN],
                                 start=st, stop=sp)
                nc.tensor.matmul(av1[:, n0:SPAN], vnat(1, jt),
                                 p[:, SPAN + n0:2 * SPAN], start=st, stop=sp)

            emit_sc(0)
            if njt > 1:
                emit_sc(1)
            for jt in range(njt):
                emit_pav(jt)
            finalize(s, 0, av0)
            finalize(s, 1, av1)

        def attn2(s):
            njt = CPS * (s + 1)
            av2 = ps_av.tile([HD + 1, SPAN], F32, tag="ps_av", name=f"av2_{s}")
            live = {}

            def emit_sc(jt):
                c_d = jt - CPS * s
                n0 = max(c_d, 0) * P
                ns_k, ck = jt // CPS, jt % CPS
                sc = ps.tile([P, 2 * SPAN], F32, tag="ps", name=f"sc2_{s}_{jt}")
                nc.tensor.matmul(sc[:, n0:SPAN],
                                 kTz[2][ns_k][:, ck * P:(ck + 1) * P],
                                 qT2z[s][:, n0:SPAN], start=True, stop=True)
                live[jt] = (sc, n0, c_d >= 0)

            def emit_pav(jt):
                sc, n0, diag = live.pop(jt)
                p = ppool.tile([P, 2 * SPAN], dt_p, tag="p", name=f"p2_{s}_{jt}")
                nc.scalar.activation(p[:, n0:SPAN], sc[:, n0:SPAN], EXP)
                if diag:
                    nc.vector.tensor_mul(p[:, n0:n0 + P], p[:, n0:n0 + P],
                                         tri16[:])
                drain_even()
                if jt + 2 < njt and jt + 2 not in live:
                    emit_sc(jt + 2)
                pavs_left[0] -= 1
                nc.tensor.matmul(av2[:, n0:SPAN], vnat(2, jt), p[:, n0:SPAN],
                                 start=(jt == 0), stop=(jt == njt - 1))

            emit_sc(0)
            if njt > 1:
                emit_sc(1)
            for jt in range(njt):
                emit_pav(jt)
            finalize(s, 2, av2)

        # spans 0 and 1 project standalone (warmup mid-fill covers the
        # kt0-2 -> kt3-5 x-DMA boundary of the first chunk); spans 2 and 3
        # interleave into the attention streams of spans 0 and 1, which are
        # small and latency-chained — the proj bursts keep the PE duty cycle
        # high so the HAM clock stays at 2.4 GHz
        for ns0 in range(2):
            first = ns0 == 0
            for (moff, msz, what) in m_chunks:
                chunk_op(ns0, moff, msz, what,
                         midfill=WARMUP_MID if first else 0)
                first = False
            for c in range(CPS):
                transp01_op(ns0, c)
                transp2_op(ns0, c)
        for s in range(NS):
            pending.extend(proj_ops(s + 2) if s + 2 < NS else [])
            pavs_left[0] = 2 * CPS * (s + 1)
            attn01(s)
            attn2(s)
            while pending:
                pending.pop(0)()


_NC_CACHE = {}


def _get_module(dt_proj=DT_PROJ, dt_p=DT_P):
    key = (dt_proj, dt_p)
    if key not in _NC_CACHE:
        nc = bacc.Bacc("TRN2", target_bir_lowering=False, debug=False)
        with tile.TileContext(nc) as tc:
            _build(nc, tc, dt_proj, dt_p)
        nc.compile()
        _NC_CACHE[key] = nc
    return _NC_CACHE[key]


def _in_maps(x, Wq, Wk, Wv):
    maps = []
    xT = [np.ascontiguousarray(
        x[b].T.reshape(KT, P, NS, SPAN).transpose(1, 2, 0, 3).reshape(P, -1))
        for b in range(B)]
    WqT, WkT, WvT = Wq.T, Wk.T, Wv.T
    for c in range(N_CORES):
        bc, g = divmod(c, N_CORES // B)
        s0 = g * DL
        wcomb = np.concatenate([
            WqT[:, s0:s0 + P], WkT[:, s0:s0 + P], WvT[:, s0:s0 + P],
            WkT[:, s0 + P:s0 + DL], WvT[:, s0 + P:s0 + DL],
            WqT[:, s0 + P:s0 + DL]], axis=1)
        wpk = np.ascontiguousarray(
            wcomb.reshape(KT, P, 3 * DL).transpose(1, 0, 2).reshape(P, -1))
        maps.append({
            "xt": xT[bc],
            "wc": wpk,
        })
    return maps


def kernel(x, Wq, Wk, Wv, _trace=False, _tmpdir=None, **_kw):
    x = np.asarray(x, dtype=np.float32)
    Wq = np.asarray(Wq, dtype=np.float32)
    Wk = np.asarray(Wk, dtype=np.float32)
    Wv = np.asarray(Wv, dtype=np.float32)
    assert x.shape == (B, N, D) and Wq.shape == (D, D)

    nc = _get_module()
    res = bass_utils.run_bass_kernel_spmd(
        nc, _in_maps(x, Wq, Wk, Wv), core_ids=list(range(N_CORES)),
        trace=_trace, tmpdir=_tmpdir)
    out = np.empty((B, N, D), np.float32)
    for c in range(N_CORES):
        bc, g = divmod(c, N_CORES // B)
        oT = res.results[c]["o"].astype(np.float64)
        for h in range(HL):
            blk = oT[h * (HD + 1):h * (HD + 1) + HD, :]
            den = oT[h * (HD + 1) + HD, :]
            out[bc, :, g * DL + h * HD:g * DL + (h + 1) * HD] = \
                (blk / den).T.astype(np.float32)
    if _trace:
        return out, res
    return out
